# revision 3
# baseline (speedup 1.0000x reference)
"""MinLSTM fused kernel for Trainium2 (8 NeuronCores, SPMD).

Math: the reference applies cumlogsumexp over the sequence but only the LAST
timestep feeds the output head, so the scan collapses to a single logsumexp
reduction over sequence:

    log_h_last = log_f[S-1] + log(0.5 + sum_s exp(diff_s + log_g(h_s)))
    out = exp(log_h_last) @ w_out.T + b_out

with diff = softplus(-f) - softplus(-i) and per-token term

    exp(diff + log_g(h)) = (1 + e^{-f}) * sigmoid(i) * g(h)
                         = 1/4 * (1+e^{-f}) * (1+tanh(i/2)) * (1+max(2h, tanh(h/2)))

which needs only {exp, tanh} — both in the ACT `exp_and_others` table set
(single table load). The device computes, per core, the partial sum over its
4096 tokens of that product for each of the 1024 hidden channels, fused with
the z = x @ w_in.T matmul (bf16, fp32 PSUM accumulation). The host combines
partials, applies the exact last-token correction in fp64, and runs the tiny
[4,1024]x[1024,1024] output head.

Sharding: data-parallel over flattened (batch, seq) tokens — core c takes
tokens [c*4096, (c+1)*4096), i.e. batch c//2, sequence half c%2. The sum over
seq is order-independent, so partials combine by addition on host.
"""

from contextlib import ExitStack

import ml_dtypes
import numpy as np

B, S, D, H = 4, 8192, 1024, 1024
N_CORES = 8
TOK = B * S // N_CORES  # 4096 tokens per core
TB = 512                # token block (matmul moving free dim)
NTB = TOK // TB         # 8
KC = D // 128           # 8 contraction chunks
JC = H // 128           # 8 hidden-channel chunks per gate

_CACHE = {}


def _build_nc():
    import concourse.bacc as bacc
    import concourse.mybir as mybir
    import concourse.tile as tile

    dt = mybir.dt
    AF = mybir.ActivationFunctionType
    ALU = mybir.AluOpType

    nc = bacc.Bacc("TRN2", target_bir_lowering=False)
    xT = nc.dram_tensor("xt", (D, TOK), dt.bfloat16, kind="ExternalInput")
    wT = nc.dram_tensor("wt", (D, 3 * H), dt.bfloat16, kind="ExternalInput")
    out_sums = nc.dram_tensor("sums", (JC, 128), dt.float32, kind="ExternalOutput")

    with tile.TileContext(nc) as tc, ExitStack() as ctx:
        wpool = ctx.enter_context(tc.tile_pool(name="w", bufs=1))
        xpool = ctx.enter_context(tc.tile_pool(name="x", bufs=3))
        gpool = ctx.enter_context(tc.tile_pool(name="g", bufs=3))
        spool = ctx.enter_context(tc.tile_pool(name="s", bufs=1))
        psum = ctx.enter_context(tc.tile_pool(name="psum", bufs=2, space="PSUM"))

        w_sb = []
        for kc in range(KC):
            wt_t = wpool.tile([128, 3 * H], dt.bfloat16, tag=f"w{kc}")
            nc.sync.dma_start(wt_t[:], wT[kc * 128 : (kc + 1) * 128, :])
            w_sb.append(wt_t)

        slab = spool.tile([128, JC, NTB], dt.float32)

        xT_r = xT[:].rearrange("(kc p) s -> p kc s", p=128)

        for tb in range(NTB):
            x_sb = xpool.tile([128, KC, TB], dt.bfloat16, tag="x")
            nc.sync.dma_start(x_sb[:], xT_r[:, :, tb * TB : (tb + 1) * TB])
            for j in range(JC):
                ps = []
                for g in range(3):
                    pt = psum.tile([128, TB], dt.float32, tag=f"ps{g}")
                    hs = g * H + j * 128
                    for kc in range(KC):
                        nc.tensor.matmul(
                            pt[:],
                            w_sb[kc][:, hs : hs + 128],
                            x_sb[:, kc, :],
                            start=(kc == 0),
                            stop=(kc == KC - 1),
                        )
                    ps.append(pt)
                a = gpool.tile([128, TB], dt.bfloat16, tag="a")
                ti = gpool.tile([128, TB], dt.bfloat16, tag="ti")
                th = gpool.tile([128, TB], dt.bfloat16, tag="th")
                nc.scalar.activation(a[:], ps[0][:], AF.Exp, scale=-1.0)
                nc.scalar.activation(ti[:], ps[1][:], AF.Tanh, scale=0.5)
                nc.scalar.activation(th[:], ps[2][:], AF.Tanh, scale=0.5)
                # m1 = max(2h, tanh(h/2));  p = (1+tanh(i/2)) * (1+m1)
                m1 = gpool.tile([128, TB], dt.bfloat16, tag="m1")
                nc.vector.scalar_tensor_tensor(
                    m1[:], ps[2][:], 2.0, th[:], op0=ALU.mult, op1=ALU.max
                )
                w2 = gpool.tile([128, TB], dt.bfloat16, tag="w2")
                nc.vector.tensor_scalar_add(w2[:], m1[:], 1.0)
                p = gpool.tile([128, TB], dt.bfloat16, tag="p")
                nc.vector.scalar_tensor_tensor(
                    p[:], ti[:], 1.0, w2[:], op0=ALU.add, op1=ALU.mult
                )
                # t = (1+e^{-f}) * p, accumulated over the 512 tokens
                t = gpool.tile([128, TB], dt.bfloat16, tag="t")
                nc.vector.scalar_tensor_tensor(
                    t[:],
                    a[:],
                    1.0,
                    p[:],
                    op0=ALU.add,
                    op1=ALU.mult,
                    accum_out=slab[:, j, tb : tb + 1],
                )

        red = spool.tile([128, JC], dt.float32)
        nc.vector.tensor_reduce(red[:], slab[:], axis=mybir.AxisListType.X, op=ALU.add)
        nc.sync.dma_start(out_sums[:].rearrange("j h -> h j"), red[:])

    nc.compile()
    return nc


def _get_nc():
    if "nc" not in _CACHE:
        _CACHE["nc"] = _build_nc()
    return _CACHE["nc"]


def _softplus(v):
    return np.log1p(np.exp(-np.abs(v))) + np.maximum(v, 0.0)


def kernel(x, w_in, w_out, b_out, _return_results=False, _trace=False):
    from concourse.bass_utils import run_bass_kernel_spmd

    x = np.asarray(x)
    w_in = np.asarray(w_in)
    w_out = np.asarray(w_out)
    b_out = np.asarray(b_out)

    bf16 = ml_dtypes.bfloat16
    xf = x.reshape(B * S, D)
    wT = np.ascontiguousarray(w_in.T).astype(bf16)  # [D, 3H]
    in_maps = []
    for c in range(N_CORES):
        xs = xf[c * TOK : (c + 1) * TOK]  # [TOK, D]
        in_maps.append({"xt": np.ascontiguousarray(xs.T).astype(bf16), "wt": wT})

    nc = _get_nc()
    res = run_bass_kernel_spmd(
        nc, in_maps, core_ids=list(range(N_CORES)), trace=_trace
    )

    parts = [np.asarray(r["sums"]).reshape(H).astype(np.float64) for r in res.results]
    Ssum = np.stack([parts[2 * b] + parts[2 * b + 1] for b in range(B)]) * 0.25

    # exact last-token factor in fp64 (host): log_f[S-1] = -softplus(diff[S-1])
    z_last = x[:, -1, :].astype(np.float64) @ w_in.astype(np.float64).T
    f_l, i_l = z_last[:, :H], z_last[:, H : 2 * H]
    diff_l = _softplus(-f_l) - _softplus(-i_l)
    h_last = np.exp(-_softplus(diff_l) + np.log(0.5 + Ssum))
    out = (h_last @ w_out.astype(np.float64).T + b_out.astype(np.float64)).astype(
        np.float32
    )
    if _return_results:
        return out, res
    return out


# revision 5
# speedup vs baseline: 1.0436x; 1.0436x over previous
"""MinLSTM fused kernel for Trainium2 (8 NeuronCores, SPMD).

Math: the reference applies cumlogsumexp over the sequence but only the LAST
timestep feeds the output head, so the scan collapses to a single logsumexp
reduction over sequence:

    log_h_last = log_f[S-1] + log(0.5 + sum_s exp(diff_s + log_g(h_s)))
    out = exp(log_h_last) @ w_out.T + b_out

with diff = softplus(-f) - softplus(-i) and per-token term

    exp(diff + log_g(h)) = (1 + e^{-f}) * sigmoid(i) * g(h)
                         = 1/4 * (1+e^{-f}) * (1+tanh(i/2)) * (1+max(2h, tanh(h/2)))

which needs only {exp, tanh} — both in the ACT `exp_and_others` table set
(single table load). The device computes, per core, the partial sum over its
4096 tokens of that product for each of the 1024 hidden channels, fused with
the z = x @ w_in.T matmul (bf16, fp32 PSUM accumulation). The host combines
partials, applies the exact last-token correction in fp64, and runs the tiny
[4,1024]x[1024,1024] output head.

Sharding: data-parallel over flattened (batch, seq) tokens — core c takes
tokens [c*4096, (c+1)*4096), i.e. batch c//2, sequence half c%2. The sum over
seq is order-independent, so partials combine by addition on host.
"""

from contextlib import ExitStack

import ml_dtypes
import numpy as np

B, S, D, H = 4, 8192, 1024, 1024
N_CORES = 8
TOK = B * S // N_CORES  # 4096 tokens per core
TB = 512                # token block (matmul moving free dim)
NTB = TOK // TB         # 8
KC = D // 128           # 8 contraction chunks
JC = H // 128           # 8 hidden-channel chunks per gate

_CACHE = {}


def _build_nc():
    import concourse.bacc as bacc
    import concourse.mybir as mybir
    import concourse.tile as tile

    dt = mybir.dt
    AF = mybir.ActivationFunctionType
    ALU = mybir.AluOpType

    nc = bacc.Bacc("TRN2", target_bir_lowering=False)
    xT = nc.dram_tensor("xt", (D, TOK), dt.bfloat16, kind="ExternalInput")
    wT = nc.dram_tensor("wt", (D, 3 * H), dt.bfloat16, kind="ExternalInput")
    out_sums = nc.dram_tensor("sums", (JC, 128), dt.float32, kind="ExternalOutput")

    with tile.TileContext(nc) as tc, ExitStack() as ctx:
        wpool = ctx.enter_context(tc.tile_pool(name="w", bufs=1))
        xpool = ctx.enter_context(tc.tile_pool(name="x", bufs=3))
        gpool = ctx.enter_context(tc.tile_pool(name="g", bufs=3))
        spool = ctx.enter_context(tc.tile_pool(name="s", bufs=1))
        psum = ctx.enter_context(tc.tile_pool(name="psum", bufs=2, space="PSUM"))

        slab = spool.tile([128, JC, NTB], dt.float32)

        xT_r = xT[:].rearrange("(kc p) s -> p kc s", p=128)
        wT_r = wT[:].rearrange("(kc p) h -> p kc h", p=128)

        # preload x for tb=0 first, then stream w in 128-column stripes in
        # the order the j-loop consumes them so the first matmul group only
        # waits for ~3 stripes (~0.8 MB) instead of the full 6 MB.
        x_first = xpool.tile([128, KC, TB], dt.bfloat16, tag="x")
        nc.sync.dma_start(x_first[:], xT_r[:, :, 0:TB])
        w_all = wpool.tile([128, KC, 3 * H], dt.bfloat16)
        for j in range(JC):
            for g in range(3):
                hs = g * H + j * 128
                nc.sync.dma_start(
                    w_all[:, :, hs : hs + 128], wT_r[:, :, hs : hs + 128]
                )

        for tb in range(NTB):
            if tb == 0:
                x_sb = x_first
            else:
                x_sb = xpool.tile([128, KC, TB], dt.bfloat16, tag="x")
                nc.sync.dma_start(x_sb[:], xT_r[:, :, tb * TB : (tb + 1) * TB])
            for j in range(JC):
                ps = []
                for g in range(3):
                    pt = psum.tile([128, TB], dt.float32, tag=f"ps{g}")
                    hs = g * H + j * 128
                    for kc in range(KC):
                        nc.tensor.matmul(
                            pt[:],
                            w_all[:, kc, hs : hs + 128],
                            x_sb[:, kc, :],
                            start=(kc == 0),
                            stop=(kc == KC - 1),
                        )
                    ps.append(pt)
                a = gpool.tile([128, TB], dt.bfloat16, tag="a")
                ti = gpool.tile([128, TB], dt.bfloat16, tag="ti")
                th = gpool.tile([128, TB], dt.bfloat16, tag="th")
                nc.scalar.activation(a[:], ps[0][:], AF.Exp, scale=-1.0)
                nc.scalar.activation(ti[:], ps[1][:], AF.Tanh, scale=0.5)
                nc.scalar.activation(th[:], ps[2][:], AF.Tanh, scale=0.5)
                # m1 = max(2h, tanh(h/2));  p = (1+tanh(i/2)) * (1+m1)
                m1 = gpool.tile([128, TB], dt.bfloat16, tag="m1")
                nc.vector.scalar_tensor_tensor(
                    m1[:], ps[2][:], 2.0, th[:], op0=ALU.mult, op1=ALU.max
                )
                w2 = gpool.tile([128, TB], dt.bfloat16, tag="w2")
                nc.vector.tensor_scalar_add(w2[:], m1[:], 1.0)
                p = gpool.tile([128, TB], dt.bfloat16, tag="p")
                nc.vector.scalar_tensor_tensor(
                    p[:], ti[:], 1.0, w2[:], op0=ALU.add, op1=ALU.mult
                )
                # t = (1+e^{-f}) * p, accumulated over the 512 tokens
                t = gpool.tile([128, TB], dt.bfloat16, tag="t")
                nc.vector.scalar_tensor_tensor(
                    t[:],
                    a[:],
                    1.0,
                    p[:],
                    op0=ALU.add,
                    op1=ALU.mult,
                    accum_out=slab[:, j, tb : tb + 1],
                )

        red = spool.tile([128, JC], dt.float32)
        nc.vector.tensor_reduce(red[:], slab[:], axis=mybir.AxisListType.X, op=ALU.add)
        nc.sync.dma_start(out_sums[:].rearrange("j h -> h j"), red[:])

    nc.compile()
    return nc


def _get_nc():
    if "nc" not in _CACHE:
        _CACHE["nc"] = _build_nc()
    return _CACHE["nc"]


def _softplus(v):
    return np.log1p(np.exp(-np.abs(v))) + np.maximum(v, 0.0)


def kernel(x, w_in, w_out, b_out, _return_results=False, _trace=False):
    from concourse.bass_utils import run_bass_kernel_spmd

    x = np.asarray(x)
    w_in = np.asarray(w_in)
    w_out = np.asarray(w_out)
    b_out = np.asarray(b_out)

    bf16 = ml_dtypes.bfloat16
    xf = x.reshape(B * S, D)
    wT = np.ascontiguousarray(w_in.T).astype(bf16)  # [D, 3H]
    in_maps = []
    for c in range(N_CORES):
        xs = xf[c * TOK : (c + 1) * TOK]  # [TOK, D]
        in_maps.append({"xt": np.ascontiguousarray(xs.T).astype(bf16), "wt": wT})

    nc = _get_nc()
    res = run_bass_kernel_spmd(
        nc, in_maps, core_ids=list(range(N_CORES)), trace=_trace
    )

    parts = [np.asarray(r["sums"]).reshape(H).astype(np.float64) for r in res.results]
    Ssum = np.stack([parts[2 * b] + parts[2 * b + 1] for b in range(B)]) * 0.25

    # exact last-token factor in fp64 (host): log_f[S-1] = -softplus(diff[S-1])
    z_last = x[:, -1, :].astype(np.float64) @ w_in.astype(np.float64).T
    f_l, i_l = z_last[:, :H], z_last[:, H : 2 * H]
    diff_l = _softplus(-f_l) - _softplus(-i_l)
    h_last = np.exp(-_softplus(diff_l) + np.log(0.5 + Ssum))
    out = (h_last @ w_out.astype(np.float64).T + b_out.astype(np.float64)).astype(
        np.float32
    )
    if _return_results:
        return out, res
    return out


# revision 6
# speedup vs baseline: 1.6967x; 1.6259x over previous
"""MinLSTM fused kernel for Trainium2 (8 NeuronCores, SPMD).

Math: the reference applies cumlogsumexp over the sequence but only the LAST
timestep feeds the output head, so the scan collapses to a single logsumexp
reduction over sequence:

    log_h_last = log_f[S-1] + log(0.5 + sum_s exp(diff_s + log_g(h_s)))
    out = exp(log_h_last) @ w_out.T + b_out

with diff = softplus(-f) - softplus(-i) and per-token term

    exp(diff + log_g(h)) = (1 + e^{-f}) * sigmoid(i) * g(h)
                         = 1/4 * (1+e^{-f}) * (1+tanh(i/2)) * (1+max(2h, tanh(h/2)))

which needs only {exp, tanh} — both in the ACT `exp_and_others` table set
(single table load). The device computes, per core, the partial sum over its
4096 tokens of that product for each of the 1024 hidden channels, fused with
the z = x @ w_in.T matmul (fp8 DoubleRow or bf16, fp32 PSUM accumulation).
The host combines partials, applies the exact last-token correction in fp64,
and runs the tiny [4,1024]x[1024,1024] output head.

Sharding: data-parallel over flattened (batch, seq) tokens — core c takes
tokens [c*4096, (c+1)*4096), i.e. batch c//2, sequence half c%2. The sum over
seq is order-independent, so partials combine by addition on host.
"""

from contextlib import ExitStack

import ml_dtypes
import numpy as np

B, S, D, H = 4, 8192, 1024, 1024
N_CORES = 8
TOK = B * S // N_CORES  # 4096 tokens per core
TB = 512                # token block (matmul moving free dim / PSUM bank)
NTB = TOK // TB         # 8
KC = D // 128           # 8 contraction chunks of 128
JC = H // 128           # 8 hidden-channel chunks per gate

USE_FP8 = True
WSCALE = 64.0           # w pre-scale so fp8 w values sit in the normal range

_CACHE = {}


def _build_nc(use_fp8):
    import concourse.bacc as bacc
    import concourse.mybir as mybir
    import concourse.tile as tile

    dt = mybir.dt
    AF = mybir.ActivationFunctionType
    ALU = mybir.AluOpType

    in_dt = dt.float8e4 if use_fp8 else dt.bfloat16
    inv = 1.0 / WSCALE if use_fp8 else 1.0

    nc = bacc.Bacc("TRN2", target_bir_lowering=False)
    xT = nc.dram_tensor("xt", (D, TOK), in_dt, kind="ExternalInput")
    wT = nc.dram_tensor("wt", (D, 3 * H), in_dt, kind="ExternalInput")
    out_sums = nc.dram_tensor("sums", (JC, 128), dt.float32, kind="ExternalOutput")

    with tile.TileContext(nc) as tc, ExitStack() as ctx:
        wpool = ctx.enter_context(tc.tile_pool(name="w", bufs=1))
        xpool = ctx.enter_context(tc.tile_pool(name="x", bufs=3))
        gpool = ctx.enter_context(tc.tile_pool(name="g", bufs=3))
        spool = ctx.enter_context(tc.tile_pool(name="s", bufs=1))
        psum = ctx.enter_context(tc.tile_pool(name="psum", bufs=2, space="PSUM"))

        slab = spool.tile([128, JC, NTB], dt.float32)

        xT_r = xT[:].rearrange("(kc p) s -> p kc s", p=128)
        wT_r = wT[:].rearrange("(kc p) h -> p kc h", p=128)

        # preload x for tb=0 first, then stream w in 128-column stripes in
        # the order the j-loop consumes them so the first matmul group only
        # waits for ~3 stripes instead of the full weight matrix.
        x_first = xpool.tile([128, KC, TB], in_dt, tag="x")
        nc.sync.dma_start(x_first[:], xT_r[:, :, 0:TB])
        w_all = wpool.tile([128, KC, 3 * H], in_dt)
        for j in range(JC):
            for g in range(3):
                hs = g * H + j * 128
                nc.sync.dma_start(
                    w_all[:, :, hs : hs + 128], wT_r[:, :, hs : hs + 128]
                )

        for tb in range(NTB):
            if tb == 0:
                x_sb = x_first
            else:
                x_sb = xpool.tile([128, KC, TB], in_dt, tag="x")
                nc.sync.dma_start(x_sb[:], xT_r[:, :, tb * TB : (tb + 1) * TB])
            for j in range(JC):
                ps = []
                for g in range(3):
                    pt = psum.tile([128, TB], dt.float32, tag=f"ps{g}")
                    hs = g * H + j * 128
                    if use_fp8:
                        for kb in range(KC // 2):
                            nc.tensor.matmul(
                                pt[:],
                                w_all[:, 2 * kb : 2 * kb + 2, hs : hs + 128],
                                x_sb[:, 2 * kb : 2 * kb + 2, :],
                                start=(kb == 0),
                                stop=(kb == KC // 2 - 1),
                                perf_mode=mybir.MatmulPerfMode.DoubleRow,
                            )
                    else:
                        for kc in range(KC):
                            nc.tensor.matmul(
                                pt[:],
                                w_all[:, kc, hs : hs + 128],
                                x_sb[:, kc, :],
                                start=(kc == 0),
                                stop=(kc == KC - 1),
                            )
                    ps.append(pt)
                a = gpool.tile([128, TB], dt.bfloat16, tag="a")
                ti = gpool.tile([128, TB], dt.bfloat16, tag="ti")
                th = gpool.tile([128, TB], dt.bfloat16, tag="th")
                nc.scalar.activation(a[:], ps[0][:], AF.Exp, scale=-inv)
                nc.scalar.activation(ti[:], ps[1][:], AF.Tanh, scale=0.5 * inv)
                nc.scalar.activation(th[:], ps[2][:], AF.Tanh, scale=0.5 * inv)
                # m1 = max(2h, tanh(h/2));  p = (1+tanh(i/2)) * (1+m1)
                m1 = gpool.tile([128, TB], dt.bfloat16, tag="m1")
                nc.vector.scalar_tensor_tensor(
                    m1[:], ps[2][:], 2.0 * inv, th[:], op0=ALU.mult, op1=ALU.max
                )
                w2 = gpool.tile([128, TB], dt.bfloat16, tag="w2")
                nc.vector.tensor_scalar_add(w2[:], m1[:], 1.0)
                p = gpool.tile([128, TB], dt.bfloat16, tag="p")
                nc.vector.scalar_tensor_tensor(
                    p[:], ti[:], 1.0, w2[:], op0=ALU.add, op1=ALU.mult
                )
                # t = (1+e^{-f}) * p, accumulated over the 512 tokens
                t = gpool.tile([128, TB], dt.bfloat16, tag="t")
                nc.vector.scalar_tensor_tensor(
                    t[:],
                    a[:],
                    1.0,
                    p[:],
                    op0=ALU.add,
                    op1=ALU.mult,
                    accum_out=slab[:, j, tb : tb + 1],
                )

        red = spool.tile([128, JC], dt.float32)
        nc.vector.tensor_reduce(red[:], slab[:], axis=mybir.AxisListType.X, op=ALU.add)
        nc.sync.dma_start(out_sums[:].rearrange("j h -> h j"), red[:])

    nc.compile()
    return nc


def _get_nc():
    key = ("fp8" if USE_FP8 else "bf16")
    if key not in _CACHE:
        _CACHE[key] = _build_nc(USE_FP8)
    return _CACHE[key]


def _softplus(v):
    return np.log1p(np.exp(-np.abs(v))) + np.maximum(v, 0.0)


def kernel(x, w_in, w_out, b_out, _return_results=False, _trace=False):
    from concourse.bass_utils import run_bass_kernel_spmd

    x = np.asarray(x)
    w_in = np.asarray(w_in)
    w_out = np.asarray(w_out)
    b_out = np.asarray(b_out)

    if USE_FP8:
        cast_dt = ml_dtypes.float8_e4m3  # TRN FP8_EXP4: max ±240, inf above

        def cast(a):
            return np.clip(a, -240.0, 240.0).astype(cast_dt)

        wT = cast(np.ascontiguousarray(w_in.T) * WSCALE)  # [D, 3H]
    else:
        cast_dt = ml_dtypes.bfloat16

        def cast(a):
            return a.astype(cast_dt)

        wT = cast(np.ascontiguousarray(w_in.T))

    xf = x.reshape(B * S, D)
    in_maps = []
    for c in range(N_CORES):
        xs = xf[c * TOK : (c + 1) * TOK]  # [TOK, D]
        in_maps.append({"xt": cast(np.ascontiguousarray(xs.T)), "wt": wT})

    nc = _get_nc()
    res = run_bass_kernel_spmd(
        nc, in_maps, core_ids=list(range(N_CORES)), trace=_trace
    )

    parts = [np.asarray(r["sums"]).reshape(H).astype(np.float64) for r in res.results]
    Ssum = np.stack([parts[2 * b] + parts[2 * b + 1] for b in range(B)]) * 0.25

    # exact last-token factor in fp64 (host): log_f[S-1] = -softplus(diff[S-1])
    z_last = x[:, -1, :].astype(np.float64) @ w_in.astype(np.float64).T
    f_l, i_l = z_last[:, :H], z_last[:, H : 2 * H]
    diff_l = _softplus(-f_l) - _softplus(-i_l)
    h_last = np.exp(-_softplus(diff_l) + np.log(0.5 + Ssum))
    out = (h_last @ w_out.astype(np.float64).T + b_out.astype(np.float64)).astype(
        np.float32
    )
    if _return_results:
        return out, res
    return out


# revision 8
# speedup vs baseline: 1.7651x; 1.0403x over previous
"""MinLSTM fused kernel for Trainium2 (8 NeuronCores, SPMD).

Math: the reference applies cumlogsumexp over the sequence but only the LAST
timestep feeds the output head, so the scan collapses to a single logsumexp
reduction over sequence:

    log_h_last = log_f[S-1] + log(0.5 + sum_s exp(diff_s + log_g(h_s)))
    out = exp(log_h_last) @ w_out.T + b_out

with diff = softplus(-f) - softplus(-i) and per-token term

    exp(diff + log_g(h)) = (1 + e^{-f}) * sigmoid(i) * g(h)
                         = 1/4 * (1+e^{-f}) * (1+tanh(i/2)) * (1+max(2h, tanh(h/2)))

which needs only {exp, tanh} — both in the ACT `exp_and_others` table set
(single table load). The device computes, per core, the partial sum over its
4096 tokens of that product for each of the 1024 hidden channels, fused with
the z = x @ w_in.T matmul (fp8 DoubleRow or bf16, fp32 PSUM accumulation).
The host combines partials, applies the exact last-token correction in fp64,
and runs the tiny [4,1024]x[1024,1024] output head.

Sharding: data-parallel over flattened (batch, seq) tokens — core c takes
tokens [c*4096, (c+1)*4096), i.e. batch c//2, sequence half c%2. The sum over
seq is order-independent, so partials combine by addition on host.
"""

from contextlib import ExitStack

import ml_dtypes
import numpy as np

B, S, D, H = 4, 8192, 1024, 1024
N_CORES = 8
TOK = B * S // N_CORES  # 4096 tokens per core
TB = 512                # token block (matmul moving free dim / PSUM bank)
NTB = TOK // TB         # 8
KC = D // 128           # 8 contraction chunks of 128
JC = H // 128           # 8 hidden-channel chunks per gate

USE_FP8 = True
WSCALE = 64.0           # w pre-scale so fp8 w values sit in the normal range

_CACHE = {}


def _build_nc(use_fp8):
    import concourse.bacc as bacc
    import concourse.mybir as mybir
    import concourse.tile as tile

    dt = mybir.dt
    AF = mybir.ActivationFunctionType
    ALU = mybir.AluOpType

    in_dt = dt.float8e4 if use_fp8 else dt.bfloat16
    inv = 1.0 / WSCALE if use_fp8 else 1.0

    nc = bacc.Bacc("TRN2", target_bir_lowering=False)
    xT = nc.dram_tensor("xt", (D, TOK), in_dt, kind="ExternalInput")
    wT = nc.dram_tensor("wt", (D, 3 * H), in_dt, kind="ExternalInput")
    out_sums = nc.dram_tensor("sums", (JC, 128), dt.float32, kind="ExternalOutput")

    with tile.TileContext(nc) as tc, ExitStack() as ctx:
        wpool = ctx.enter_context(tc.tile_pool(name="w", bufs=1))
        xpool = ctx.enter_context(tc.tile_pool(name="x", bufs=3))
        gpool = ctx.enter_context(tc.tile_pool(name="g", bufs=3))
        spool = ctx.enter_context(tc.tile_pool(name="s", bufs=1))
        psum = ctx.enter_context(tc.tile_pool(name="psum", bufs=2, space="PSUM"))

        slab = spool.tile([128, JC, NTB], dt.float32)

        xT_r = xT[:].rearrange("(kc p) s -> p kc s", p=128)
        wT_r = wT[:].rearrange("(kc p) h -> p kc h", p=128)

        # preload x for tb=0 first, then stream w in j-ordered stripes (one
        # DMA per j covers the 128-column slice of all 3 gates via a strided
        # view) so the first matmul group only waits for ~1/8 of the weights.
        # w DMAs go on the idle GpSimd queue so their issue cost doesn't
        # serialize against the x loads on Sync.
        x_first = xpool.tile([128, KC, TB], in_dt, tag="x")
        nc.sync.dma_start(x_first[:], xT_r[:, :, 0:TB])
        w_all = wpool.tile([128, KC, 3 * H], in_dt)
        for j in range(JC):
            for g in range(3):
                hs = g * H + j * 128
                nc.gpsimd.dma_start(
                    w_all[:, :, hs : hs + 128], wT_r[:, :, hs : hs + 128]
                )

        for tb in range(NTB):
            if tb == 0:
                x_sb = x_first
            else:
                x_sb = xpool.tile([128, KC, TB], in_dt, tag="x")
                nc.sync.dma_start(x_sb[:], xT_r[:, :, tb * TB : (tb + 1) * TB])
            for j in range(JC):
                ps = []
                for g in range(3):
                    pt = psum.tile([128, TB], dt.float32, tag=f"ps{g}")
                    hs = g * H + j * 128
                    if use_fp8:
                        for kb in range(KC // 2):
                            nc.tensor.matmul(
                                pt[:],
                                w_all[:, 2 * kb : 2 * kb + 2, hs : hs + 128],
                                x_sb[:, 2 * kb : 2 * kb + 2, :],
                                start=(kb == 0),
                                stop=(kb == KC // 2 - 1),
                                perf_mode=mybir.MatmulPerfMode.DoubleRow,
                            )
                    else:
                        for kc in range(KC):
                            nc.tensor.matmul(
                                pt[:],
                                w_all[:, kc, hs : hs + 128],
                                x_sb[:, kc, :],
                                start=(kc == 0),
                                stop=(kc == KC - 1),
                            )
                    ps.append(pt)
                a = gpool.tile([128, TB], dt.bfloat16, tag="a")
                ti = gpool.tile([128, TB], dt.bfloat16, tag="ti")
                th = gpool.tile([128, TB], dt.bfloat16, tag="th")
                nc.scalar.activation(a[:], ps[0][:], AF.Exp, scale=-inv)
                nc.scalar.activation(ti[:], ps[1][:], AF.Tanh, scale=0.5 * inv)
                nc.scalar.activation(th[:], ps[2][:], AF.Tanh, scale=0.5 * inv)
                # m1 = max(2h, tanh(h/2));  p = (1+tanh(i/2)) * (1+m1)
                m1 = gpool.tile([128, TB], dt.bfloat16, tag="m1")
                nc.vector.scalar_tensor_tensor(
                    m1[:], ps[2][:], 2.0 * inv, th[:], op0=ALU.mult, op1=ALU.max
                )
                w2 = gpool.tile([128, TB], dt.bfloat16, tag="w2")
                nc.vector.tensor_scalar_add(w2[:], m1[:], 1.0)
                p = gpool.tile([128, TB], dt.bfloat16, tag="p")
                nc.vector.scalar_tensor_tensor(
                    p[:], ti[:], 1.0, w2[:], op0=ALU.add, op1=ALU.mult
                )
                # t = (1+e^{-f}) * p, accumulated over the 512 tokens
                t = gpool.tile([128, TB], dt.bfloat16, tag="t")
                nc.vector.scalar_tensor_tensor(
                    t[:],
                    a[:],
                    1.0,
                    p[:],
                    op0=ALU.add,
                    op1=ALU.mult,
                    accum_out=slab[:, j, tb : tb + 1],
                )

        red = spool.tile([128, JC], dt.float32)
        nc.vector.tensor_reduce(red[:], slab[:], axis=mybir.AxisListType.X, op=ALU.add)
        nc.sync.dma_start(out_sums[:].rearrange("j h -> h j"), red[:])

    nc.compile()
    return nc


def _get_nc():
    key = ("fp8" if USE_FP8 else "bf16")
    if key not in _CACHE:
        _CACHE[key] = _build_nc(USE_FP8)
    return _CACHE[key]


def _softplus(v):
    return np.log1p(np.exp(-np.abs(v))) + np.maximum(v, 0.0)


def kernel(x, w_in, w_out, b_out, _return_results=False, _trace=False):
    from concourse.bass_utils import run_bass_kernel_spmd

    x = np.asarray(x)
    w_in = np.asarray(w_in)
    w_out = np.asarray(w_out)
    b_out = np.asarray(b_out)

    if USE_FP8:
        cast_dt = ml_dtypes.float8_e4m3  # TRN FP8_EXP4: max ±240, inf above

        def cast(a):
            return np.clip(a, -240.0, 240.0).astype(cast_dt)

        wT = cast(np.ascontiguousarray(w_in.T) * WSCALE)  # [D, 3H]
    else:
        cast_dt = ml_dtypes.bfloat16

        def cast(a):
            return a.astype(cast_dt)

        wT = cast(np.ascontiguousarray(w_in.T))

    xf = x.reshape(B * S, D)
    in_maps = []
    for c in range(N_CORES):
        xs = xf[c * TOK : (c + 1) * TOK]  # [TOK, D]
        in_maps.append({"xt": cast(np.ascontiguousarray(xs.T)), "wt": wT})

    nc = _get_nc()
    res = run_bass_kernel_spmd(
        nc, in_maps, core_ids=list(range(N_CORES)), trace=_trace
    )

    parts = [np.asarray(r["sums"]).reshape(H).astype(np.float64) for r in res.results]
    Ssum = np.stack([parts[2 * b] + parts[2 * b + 1] for b in range(B)]) * 0.25

    # exact last-token factor in fp64 (host): log_f[S-1] = -softplus(diff[S-1])
    z_last = x[:, -1, :].astype(np.float64) @ w_in.astype(np.float64).T
    f_l, i_l = z_last[:, :H], z_last[:, H : 2 * H]
    diff_l = _softplus(-f_l) - _softplus(-i_l)
    h_last = np.exp(-_softplus(diff_l) + np.log(0.5 + Ssum))
    out = (h_last @ w_out.astype(np.float64).T + b_out.astype(np.float64)).astype(
        np.float32
    )
    if _return_results:
        return out, res
    return out


# revision 9
# speedup vs baseline: 1.7668x; 1.0009x over previous
"""MinLSTM fused kernel for Trainium2 (8 NeuronCores, SPMD).

Math: the reference applies cumlogsumexp over the sequence but only the LAST
timestep feeds the output head, so the scan collapses to a single logsumexp
reduction over sequence:

    log_h_last = log_f[S-1] + log(0.5 + sum_s exp(diff_s + log_g(h_s)))
    out = exp(log_h_last) @ w_out.T + b_out

with diff = softplus(-f) - softplus(-i) and per-token term

    exp(diff + log_g(h)) = (1 + e^{-f}) * sigmoid(i) * g(h)
                         = 1/4 * (1+e^{-f}) * (1+tanh(i/2)) * (1+max(2h, tanh(h/2)))

which needs only {exp, tanh} — both in the ACT `exp_and_others` table set
(single table load). The device computes, per core, the partial sum over its
4096 tokens of that product for each of the 1024 hidden channels, fused with
the z = x @ w_in.T matmul (fp8 DoubleRow or bf16, fp32 PSUM accumulation).
The host combines partials, applies the exact last-token correction in fp64,
and runs the tiny [4,1024]x[1024,1024] output head.

Sharding: data-parallel over flattened (batch, seq) tokens — core c takes
tokens [c*4096, (c+1)*4096), i.e. batch c//2, sequence half c%2. The sum over
seq is order-independent, so partials combine by addition on host.
"""

from contextlib import ExitStack

import ml_dtypes
import numpy as np

B, S, D, H = 4, 8192, 1024, 1024
N_CORES = 8
TOK = B * S // N_CORES  # 4096 tokens per core
TB = 512                # token block (matmul moving free dim / PSUM bank)
NTB = TOK // TB         # 8
KC = D // 128           # 8 contraction chunks of 128
JC = H // 128           # 8 hidden-channel chunks per gate

USE_FP8 = True
WSCALE = 64.0           # w pre-scale so fp8 w values sit in the normal range

_CACHE = {}


def _build_nc(use_fp8):
    import concourse.bacc as bacc
    import concourse.mybir as mybir
    import concourse.tile as tile

    dt = mybir.dt
    AF = mybir.ActivationFunctionType
    ALU = mybir.AluOpType

    in_dt = dt.float8e4 if use_fp8 else dt.bfloat16
    inv = 1.0 / WSCALE if use_fp8 else 1.0

    nc = bacc.Bacc("TRN2", target_bir_lowering=False)
    xT = nc.dram_tensor("xt", (D, TOK), in_dt, kind="ExternalInput")
    wT = nc.dram_tensor("wt", (D, 3 * H), in_dt, kind="ExternalInput")
    out_sums = nc.dram_tensor("sums", (JC, 128), dt.float32, kind="ExternalOutput")

    with tile.TileContext(nc) as tc, ExitStack() as ctx:
        wpool = ctx.enter_context(tc.tile_pool(name="w", bufs=1))
        xpool = ctx.enter_context(tc.tile_pool(name="x", bufs=3))
        gpool = ctx.enter_context(tc.tile_pool(name="g", bufs=3))
        spool = ctx.enter_context(tc.tile_pool(name="s", bufs=1))
        psum = ctx.enter_context(tc.tile_pool(name="psum", bufs=2, space="PSUM"))

        slab = spool.tile([128, JC, NTB], dt.float32)

        xT_r = xT[:].rearrange("(kc p) s -> p kc s", p=128)
        wT_r = wT[:].rearrange("(kc p) h -> p kc h", p=128)

        # preload x for tb=0 first, then stream w in j-ordered stripes (one
        # DMA per j covers the 128-column slice of all 3 gates via a strided
        # view) so the first matmul group only waits for ~1/8 of the weights.
        # w DMAs go on the idle GpSimd queue so their issue cost doesn't
        # serialize against the x loads on Sync.
        x_first = xpool.tile([128, KC, TB], in_dt, tag="x")
        nc.sync.dma_start(x_first[:], xT_r[:, :, 0:TB])
        w_all = wpool.tile([128, KC, 3 * H], in_dt)
        for j in range(JC):
            for g in range(3):
                hs = g * H + j * 128
                nc.gpsimd.dma_start(
                    w_all[:, :, hs : hs + 128], wT_r[:, :, hs : hs + 128]
                )

        for tb in range(NTB):
            if tb == 0:
                x_sb = x_first
            else:
                x_sb = xpool.tile([128, KC, TB], in_dt, tag="x")
                nc.sync.dma_start(x_sb[:], xT_r[:, :, tb * TB : (tb + 1) * TB])
            for j in range(JC):
                ps = []
                for g in range(3):
                    # 2+3+3 = 8 PSUM banks: later gates are drained later by
                    # the ACT/DVE chain, so they get an extra buffer.
                    pt = psum.tile(
                        [128, TB], dt.float32, tag=f"ps{g}", bufs=(2 if g == 0 else 3)
                    )
                    hs = g * H + j * 128
                    if use_fp8:
                        for kb in range(KC // 2):
                            nc.tensor.matmul(
                                pt[:],
                                w_all[:, 2 * kb : 2 * kb + 2, hs : hs + 128],
                                x_sb[:, 2 * kb : 2 * kb + 2, :],
                                start=(kb == 0),
                                stop=(kb == KC // 2 - 1),
                                perf_mode=mybir.MatmulPerfMode.DoubleRow,
                            )
                    else:
                        for kc in range(KC):
                            nc.tensor.matmul(
                                pt[:],
                                w_all[:, kc, hs : hs + 128],
                                x_sb[:, kc, :],
                                start=(kc == 0),
                                stop=(kc == KC - 1),
                            )
                    ps.append(pt)
                a = gpool.tile([128, TB], dt.bfloat16, tag="a")
                ti = gpool.tile([128, TB], dt.bfloat16, tag="ti")
                th = gpool.tile([128, TB], dt.bfloat16, tag="th")
                nc.scalar.activation(a[:], ps[0][:], AF.Exp, scale=-inv)
                nc.scalar.activation(ti[:], ps[1][:], AF.Tanh, scale=0.5 * inv)
                nc.scalar.activation(th[:], ps[2][:], AF.Tanh, scale=0.5 * inv)
                # m1 = max(2h, tanh(h/2));  p = (1+tanh(i/2)) * (1+m1)
                m1 = gpool.tile([128, TB], dt.bfloat16, tag="m1")
                nc.vector.scalar_tensor_tensor(
                    m1[:], ps[2][:], 2.0 * inv, th[:], op0=ALU.mult, op1=ALU.max
                )
                w2 = gpool.tile([128, TB], dt.bfloat16, tag="w2")
                nc.vector.tensor_scalar_add(w2[:], m1[:], 1.0)
                p = gpool.tile([128, TB], dt.bfloat16, tag="p")
                nc.vector.scalar_tensor_tensor(
                    p[:], ti[:], 1.0, w2[:], op0=ALU.add, op1=ALU.mult
                )
                # t = (1+e^{-f}) * p, accumulated over the 512 tokens
                t = gpool.tile([128, TB], dt.bfloat16, tag="t")
                nc.vector.scalar_tensor_tensor(
                    t[:],
                    a[:],
                    1.0,
                    p[:],
                    op0=ALU.add,
                    op1=ALU.mult,
                    accum_out=slab[:, j, tb : tb + 1],
                )

        red = spool.tile([128, JC], dt.float32)
        nc.vector.tensor_reduce(red[:], slab[:], axis=mybir.AxisListType.X, op=ALU.add)
        nc.sync.dma_start(out_sums[:].rearrange("j h -> h j"), red[:])

    nc.compile()
    return nc


def _get_nc():
    key = ("fp8" if USE_FP8 else "bf16")
    if key not in _CACHE:
        _CACHE[key] = _build_nc(USE_FP8)
    return _CACHE[key]


def _softplus(v):
    return np.log1p(np.exp(-np.abs(v))) + np.maximum(v, 0.0)


def kernel(x, w_in, w_out, b_out, _return_results=False, _trace=False):
    from concourse.bass_utils import run_bass_kernel_spmd

    x = np.asarray(x)
    w_in = np.asarray(w_in)
    w_out = np.asarray(w_out)
    b_out = np.asarray(b_out)

    if USE_FP8:
        cast_dt = ml_dtypes.float8_e4m3  # TRN FP8_EXP4: max ±240, inf above

        def cast(a):
            return np.clip(a, -240.0, 240.0).astype(cast_dt)

        wT = cast(np.ascontiguousarray(w_in.T) * WSCALE)  # [D, 3H]
    else:
        cast_dt = ml_dtypes.bfloat16

        def cast(a):
            return a.astype(cast_dt)

        wT = cast(np.ascontiguousarray(w_in.T))

    xf = x.reshape(B * S, D)
    in_maps = []
    for c in range(N_CORES):
        xs = xf[c * TOK : (c + 1) * TOK]  # [TOK, D]
        in_maps.append({"xt": cast(np.ascontiguousarray(xs.T)), "wt": wT})

    nc = _get_nc()
    res = run_bass_kernel_spmd(
        nc, in_maps, core_ids=list(range(N_CORES)), trace=_trace
    )

    parts = [np.asarray(r["sums"]).reshape(H).astype(np.float64) for r in res.results]
    Ssum = np.stack([parts[2 * b] + parts[2 * b + 1] for b in range(B)]) * 0.25

    # exact last-token factor in fp64 (host): log_f[S-1] = -softplus(diff[S-1])
    z_last = x[:, -1, :].astype(np.float64) @ w_in.astype(np.float64).T
    f_l, i_l = z_last[:, :H], z_last[:, H : 2 * H]
    diff_l = _softplus(-f_l) - _softplus(-i_l)
    h_last = np.exp(-_softplus(diff_l) + np.log(0.5 + Ssum))
    out = (h_last @ w_out.astype(np.float64).T + b_out.astype(np.float64)).astype(
        np.float32
    )
    if _return_results:
        return out, res
    return out
